# revision 11
# baseline (speedup 1.0000x reference)
"""3-layer GCN (GCNConv x3 + ReLU) on 8 Trainium2 NeuronCores.

Strategy (1D node partition, dst-sharded):
- Nodes padded to 50176 = 8 * 6272; core c owns dst rows [c*6272, (c+1)*6272).
- Layer tables (dis-prescaled node features, bf16) replicated in each core's
  DRAM. Layer 0 table computed on host from x; tables 1,2 produced on device
  and replicated via AllGather.
- Aggregation t = A @ h per core: edges with dst in the shard are grouped
  into 128-dst windows, chunks of 128 edges. Per chunk:
    dma_gather (4 SWDGE queues) pulls h[src] rows -> msg [128e, 128f] bf16
    PE matmul: acc[128f, 128d] += msg^T-contract-e @ S (S = one-hot dst map
    scaled by edge_weight, host-precomputed bf16)
  Window epilogue: t = acc * dis[dst] (DVE, PSUM->SBUF, bf16).
- z' = relu(t @ W + b) via PE GEMM + ACT; rescale by dis, PE-transpose to
  row-major, AllGather to all cores' next-layer tables.
- Final layer: relu output (fp32) written out per-shard; host reassembles.
"""

import math

import numpy as np
import ml_dtypes

import concourse.bacc as bacc
import concourse.mybir as mybir
import concourse.tile as tile
from concourse.bass_utils import run_bass_kernel_spmd
from concourse.masks import make_identity

N = 50000
D = 128
NC = 8
NP = 50176            # padded nodes (= 392 * 128)
SH = NP // NC         # 6272 rows per core
W = 128               # dst window width
NW = SH // W          # 49 windows per core
SPLIT = 32768         # int16 index limit -> two-bucket gather
BF16 = ml_dtypes.bfloat16


def _preprocess(x, edge_index, edge_weight):
    src = np.concatenate([edge_index[0].astype(np.int64), np.arange(N, dtype=np.int64)])
    dst = np.concatenate([edge_index[1].astype(np.int64), np.arange(N, dtype=np.int64)])
    ew = np.concatenate([edge_weight.astype(np.float32), np.ones(N, np.float32)])

    deg = np.bincount(dst, weights=ew, minlength=NP).astype(np.float32)
    dis = np.where(deg > 0, 1.0 / np.sqrt(np.maximum(deg, 1e-30)), 0.0).astype(np.float32)

    table0 = np.zeros((NP, D), np.float32)
    table0[:N] = x * dis[:N, None]
    table0 = table0.astype(BF16)

    disb = np.zeros((NC, 128, SH), np.float32)
    for c in range(NC):
        disb[c] = np.broadcast_to(dis[c * SH:(c + 1) * SH][None, :], (128, SH))

    # per-core edge partitioning
    per_core = []
    cnt = np.zeros((NC, NW, 2), np.int64)
    for c in range(NC):
        m = (dst >= c * SH) & (dst < (c + 1) * SH)
        s_c, d_c, w_c = src[m], dst[m] - c * SH, ew[m]
        b_c = (s_c >= SPLIT).astype(np.int64)
        win = d_c // W
        order = np.lexsort((d_c, b_c, win))
        s_c, d_c, w_c, b_c, win = (a[order] for a in (s_c, d_c, w_c, b_c, win))
        per_core.append((s_c, d_c, w_c, b_c, win))
        for wi in range(NW):
            wm = win == wi
            cnt[c, wi, 0] = np.count_nonzero(wm & (b_c == 0))
            cnt[c, wi, 1] = np.count_nonzero(wm & (b_c == 1))

    # chunk counts per (window, bucket): max over cores, in 128-edge units
    C = np.zeros((NW, 2), np.int64)
    for wi in range(NW):
        for b in range(2):
            C[wi, b] = int(math.ceil(cnt[:, wi, b].max() / 128.0))
    ctot = int(C.sum())
    tot_idx = ctot * 128

    idx_all = np.zeros((NC, tot_idx), np.int16)
    s_all = np.zeros((NC, ctot, 128, W), BF16)
    chunk_meta = []  # (win, bucket, chunk_base, n_chunks)
    base = 0
    for wi in range(NW):
        for b in range(2):
            nch = int(C[wi, b])
            if nch == 0:
                continue
            chunk_meta.append((wi, b, base, nch))
            base += nch
    assert base == ctot

    for c in range(NC):
        s_c, d_c, w_c, b_c, win = per_core[c]
        for (wi, b, cb, nch) in chunk_meta:
            sel = (win == wi) & (b_c == b)
            ss = s_c[sel]
            dd = d_c[sel] - wi * W
            ww = w_c[sel]
            n = len(ss)
            cap = nch * 128
            assert n <= cap
            idxs = np.zeros(cap, np.int64)
            idxs[:n] = ss - (SPLIT if b else 0)
            idx_all[c, cb * 128:(cb + nch) * 128] = idxs.astype(np.int16)
            sv = np.zeros((cap, W), np.float32)
            sv[np.arange(n), dd] = ww
            s_all[c, cb:cb + nch] = sv.reshape(nch, 128, W).astype(BF16)

    # wrapped int16 layout [128, tot_idx//16]
    idx_wrapped = np.zeros((NC, 128, tot_idx // 16), np.int16)
    for c in range(NC):
        wr = idx_all[c].reshape(-1, 16).T  # [16, cols]
        idx_wrapped[c] = np.tile(wr, (8, 1))

    # S device layout [128(e), ctot, W]
    s_dev = np.ascontiguousarray(s_all.transpose(0, 2, 1, 3))

    return dict(
        table0=table0, disb=disb, idx=idx_wrapped, s=s_dev,
        chunk_meta=chunk_meta, ctot=ctot, C=C,
    )


def _build_program(meta, Ws=None, bs=None, repeat=1):
    dt = mybir.dt
    ctot = meta["ctot"]
    chunk_meta = meta["chunk_meta"]
    idx_cols = ctot * 8

    nc = bacc.Bacc(None, target_bir_lowering=False, debug=False, num_swdge_queues=4)

    t_table0 = nc.dram_tensor("table0", [NP, D], dt.bfloat16, kind="ExternalInput")
    t_idx = nc.dram_tensor("idx", [128, idx_cols], dt.int16, kind="ExternalInput")
    t_s = nc.dram_tensor("s", [128, ctot, W], dt.bfloat16, kind="ExternalInput")
    t_disb = nc.dram_tensor("disb", [128, SH], dt.float32, kind="ExternalInput")
    t_w = [nc.dram_tensor(f"w{l}", [D, D], dt.bfloat16, kind="ExternalInput")
           for l in range(3)]
    t_b = nc.dram_tensor("bias", [128, 3], dt.float32, kind="ExternalInput")
    t_out = nc.dram_tensor("out", [128, NW, D], dt.float32, kind="ExternalOutput")

    t_tab = [t_table0,
             nc.dram_tensor("table1", [NP, D], dt.bfloat16, addr_space="Shared"),
             nc.dram_tensor("table2", [NP, D], dt.bfloat16, addr_space="Shared")]

    with tile.TileContext(nc) as tc:
        with (
            tc.tile_pool(name="const", bufs=1) as cpool,
            tc.tile_pool(name="sslab", bufs=3) as spool,
            tc.tile_pool(name="msg", bufs=4) as mpool,
            tc.tile_pool(name="work", bufs=3) as wpool,
            tc.tile_pool(name="big", bufs=1) as bigpool,
            tc.tile_pool(name="psacc", bufs=3, space="PSUM") as psacc,
            tc.tile_pool(name="psgemm", bufs=2, space="PSUM") as psgemm,
            tc.tile_pool(name="pstr", bufs=2, space="PSUM") as pstr,
            tc.tile_pool(name="dram", bufs=2, space="DRAM") as dpool,
        ):
            idx_sb = cpool.tile([128, idx_cols], dt.int16)
            nc.sync.dma_start(idx_sb[:], t_idx[:])
            disb_sb = cpool.tile([128, SH], dt.float32)
            nc.sync.dma_start(disb_sb[:], t_disb[:])
            w_sb = [cpool.tile([D, D], dt.bfloat16, name=f"wsb{l}") for l in range(3)]
            for l in range(3):
                nc.sync.dma_start(w_sb[l][:], t_w[l][:])
            b_sb = cpool.tile([128, 3], dt.float32)
            nc.sync.dma_start(b_sb[:], t_b[:])
            ident16 = cpool.tile([128, 128], dt.bfloat16)
            make_identity(nc, ident16[:])
            ident32 = cpool.tile([128, 128], dt.float32)
            make_identity(nc, ident32[:])

            t_sb = bigpool.tile([128, SH], dt.bfloat16, tag="tsb")
            stage16 = bigpool.tile([128, NW, D], dt.bfloat16, tag="stage16")
            stage32 = bigpool.tile([128, NW, D], dt.float32, tag="stage32")

            qn = [0]

            def agg_layer(table):
                # aggregation: fills t_sb[:, w*W:(w+1)*W] for every window
                for wi in range(NW):
                    acc = psacc.tile([128, W], dt.float32, tag="acc")
                    first = True
                    # find this window's chunk groups
                    groups = [g for g in chunk_meta if g[0] == wi]
                    nch_w = sum(g[3] for g in groups)
                    s_sb = spool.tile([128, nch_w, W], dt.bfloat16, tag="s")
                    first_cb = groups[0][2]
                    nc.sync.dma_start(s_sb[:], t_s[:, first_cb:first_cb + nch_w, :])
                    s_off = 0
                    for gi, (_, b, cb, nch) in enumerate(groups):
                        msg = mpool.tile([128, nch, D], dt.bfloat16, tag="msg")
                        src_ap = table[0:SPLIT, :] if b == 0 else table[SPLIT:NP, :]
                        nc.gpsimd.dma_gather(
                            msg[:], src_ap, idx_sb[:, cb * 8:(cb + nch) * 8],
                            nch * 128, nch * 128, D,
                            single_packet=False, queue_num=qn[0] % 4,
                        )
                        qn[0] += 1
                        for k in range(nch):
                            nc.tensor.matmul(
                                acc[:], msg[:, k, :], s_sb[:, s_off + k, :],
                                start=first,
                                stop=(gi == len(groups) - 1 and k == nch - 1),
                            )
                            first = False
                        s_off += nch
                    # t = acc * dis[dst]  (PSUM -> SBUF, bf16)
                    nc.vector.tensor_mul(
                        out=t_sb[:, wi * W:(wi + 1) * W],
                        in0=acc[:],
                        in1=disb_sb[:, wi * W:(wi + 1) * W],
                    )

            def dense_layer(l, last):
                # z = relu(t @ W + b); for last: fp32 to stage32, else
                # bf16 * dis to stage16 (row-major via PE transpose)
                for s0 in range(0, NW, 4):  # slabs of 4 windows = 512 cols
                    ns = min(4, NW - s0)
                    cols = ns * W
                    p2 = psgemm.tile([128, 512], dt.float32, tag="gemm")
                    nc.tensor.matmul(
                        p2[:, :cols], w_sb[l][:], t_sb[:, s0 * W:s0 * W + cols],
                        start=True, stop=True,
                    )
                    z = wpool.tile([128, 512], dt.float32, tag="z")
                    nc.scalar.activation(
                        z[:, :cols], p2[:, :cols],
                        mybir.ActivationFunctionType.Relu,
                        bias=b_sb[:, l:l + 1],
                    )
                    if not last:
                        zs = wpool.tile([128, 512], dt.bfloat16, tag="zs")
                        nc.vector.tensor_mul(
                            out=zs[:, :cols], in0=z[:, :cols],
                            in1=disb_sb[:, s0 * W:s0 * W + cols],
                        )
                        for j in range(ns):
                            pt = pstr.tile([128, 128], dt.bfloat16, tag="tr")
                            nc.tensor.transpose(
                                pt[:], zs[:, j * W:(j + 1) * W], ident16[:])
                            nc.scalar.copy(
                                out=stage16[:, s0 + j, :], in_=pt[:])
                    else:
                        for j in range(ns):
                            pt = pstr.tile([128, 128], dt.float32, tag="tr")
                            nc.tensor.transpose(
                                pt[:], z[:, j * W:(j + 1) * W], ident32[:])
                            nc.scalar.copy(
                                out=stage32[:, s0 + j, :], in_=pt[:])

            for rep in range(repeat):
              for l in range(3):
                agg_layer(t_tab[l])
                dense_layer(l, last=(l == 2))
                if l < 2:
                    agin = dpool.tile([SH, D], dt.bfloat16, tag="agin")
                    # stage16 [128, NW, D] == rows (s*128+p) of own shard
                    nc.sync.dma_start(
                        agin[:].rearrange("(s p) f -> p s f", p=128), stage16[:])
                    nc.gpsimd.collective_compute(
                        "AllGather",
                        mybir.AluOpType.bypass,
                        replica_groups=[list(range(NC))],
                        ins=[agin[:].opt()],
                        outs=[t_tab[l + 1][:].opt()],
                    )
            nc.sync.dma_start(t_out[:], stage32[:])

    nc.finalize()
    return nc


_CACHE = {}


def kernel(x, edge_index, edge_weight, W0, b0, W1, b1, W2, b2):
    x = np.asarray(x, np.float32)
    edge_index = np.asarray(edge_index)
    edge_weight = np.asarray(edge_weight, np.float32)

    meta = _preprocess(x, edge_index, edge_weight)

    Ws = [np.asarray(w, np.float32).astype(BF16) for w in (W0, W1, W2)]
    bias = np.stack([np.asarray(b, np.float32) for b in (b0, b1, b2)], axis=1)  # [128,3]

    nc = _build_program(meta)

    in_maps = _make_inmaps(meta, Ws, bias)
    res = run_bass_kernel_spmd(nc, in_maps, list(range(NC)))
    return _assemble([res.results[c]["out"] for c in range(NC)])


def _make_inmaps(meta, Ws, bias):
    in_maps = []
    for c in range(NC):
        in_maps.append({
            "table0": meta["table0"],
            "idx": meta["idx"][c],
            "s": meta["s"][c],
            "disb": meta["disb"][c],
            "w0": Ws[0], "w1": Ws[1], "w2": Ws[2],
            "bias": bias.astype(np.float32),
        })
    return in_maps


def _assemble(outs):
    out = np.zeros((NP, D), np.float32)
    for c in range(NC):
        out[c * SH:(c + 1) * SH] = outs[c].transpose(1, 0, 2).reshape(SH, D)
    return out[:N]
